# revision 5
# baseline (speedup 1.0000x reference)
"""GAT (single-head) message-passing kernel for Trainium2, 8 NeuronCores.

Math: with hidden = feat @ W + b, q1 = query[0,:128], q2 = query[0,128:],
per-edge logit l(s,d) = q1.hidden[s] + q2.hidden[d] = a[s] + bv[d] is
separable, so attention weights depend only on the (src,dst) pair.  The
whole edge aggregation collapses to dense linear algebra:
    out[d] = relu( (sum_s C[s,d] * P[s,d] * hidden[s]) / (S[d]+eps) )
    S[d]   = sum_s C[s,d] * P[s,d],   P[s,d] = exp(leaky_relu(a[s]+bv[d]))
where C is the (src,dst) edge-count matrix (self-loops folded in).
exp(leaky_relu(x)) is computed as max(exp(x), exp(0.2x)) (exp monotonic).
No per-segment max subtraction is needed: logits are O(10), exp cannot
overflow fp32, and the eps perturbation is ~1e-10 relative.

Sharding: dst blocks of 128 are round-robin assigned to the 8 cores
(core 0: blocks 0,8,16; core c: blocks c, c+8, pad).  Every core
computes hidden for all 2176 nodes (cheap) and the weighted matmul for
its own dst blocks only.  C is built on host (bincount over edge keys)
and shipped pre-sharded; node features/params are replicated.
"""

import functools
import sys

import numpy as np

N_P, N_L = 2048, 128
N = N_P + N_L            # 2176 nodes
D = 128
NB = N // 128            # 17 src blocks
DSTB = 3                 # dst blocks per core (padded)
NCORES = 8
EPS = 1e-10
NEG = 0.2

# core c -> global dst block ids (-1 = zero pad)
CORE_BLOCKS = [[0, 8, 16]] + [[c, c + 8, -1] for c in range(1, NCORES)]


def _ensure_path():
    try:
        import concourse.bass  # noqa: F401
    except ImportError:
        sys.path.insert(0, "/opt/trn_rl_repo")


@functools.lru_cache(maxsize=1)
def _program():
    _ensure_path()
    import concourse.bacc as bacc
    import concourse.tile as tile
    from concourse import mybir
    from concourse.masks import make_identity

    f32 = mybir.dt.float32
    AF = mybir.ActivationFunctionType
    OP = mybir.AluOpType

    nc = bacc.Bacc(None, target_bir_lowering=False, debug=False)

    featp = nc.dram_tensor("featp", [128, NB, 128], f32, kind="ExternalInput")
    featd = nc.dram_tensor("featd", [128, DSTB, 128], f32, kind="ExternalInput")
    ctp = nc.dram_tensor("ctp", [128, NB, DSTB * 128], f32, kind="ExternalInput")
    wp = nc.dram_tensor("wp", [128, 128], f32, kind="ExternalInput")
    qcp = nc.dram_tensor("qcp", [128, 2], f32, kind="ExternalInput")
    bcp = nc.dram_tensor("bcp", [128, 1], f32, kind="ExternalInput")
    outp = nc.dram_tensor("outp", [128, DSTB, 128], f32, kind="ExternalOutput")

    with tile.TileContext(nc) as tc:
        with tc.tile_pool(name="sb", bufs=1) as sb, \
             tc.tile_pool(name="sc", bufs=2) as sc, \
             tc.tile_pool(name="ps", bufs=3, space="PSUM") as ps, \
             tc.tile_pool(name="psa", bufs=1, space="PSUM") as psa:

            fs = sb.tile([128, NB, 128], f32, tag="fs")
            fds = sb.tile([128, DSTB, 128], f32, tag="fds")
            cts = sb.tile([128, NB, DSTB * 128], f32, tag="cts")
            ws = sb.tile([128, 128], f32, tag="ws")
            qcs = sb.tile([128, 2], f32, tag="qcs")
            bcs = sb.tile([128, 1], f32, tag="bcs")
            ident = sb.tile([128, 128], f32, tag="ident")
            ft = sb.tile([128, 128], f32, tag="ft")        # scratch feat^T block
            ht = sb.tile([128, NB, 128], f32, tag="ht")    # hidden^T blocks
            hdt = sb.tile([128, DSTB, 128], f32, tag="hdt")
            hid = sb.tile([128, NB, 129], f32, tag="hid")  # hidden blocks + ones col
            ab = sb.tile([128, NB, 2], f32, tag="ab")      # (a, bv) per node
            ab02 = sb.tile([128, NB, 2], f32, tag="ab02")  # 0.2 * (a, bv)
            bvb = sb.tile([128, DSTB * 128], f32, tag="bvb")
            q2b = sb.tile([128, 128], f32, tag="q2b")      # q2 col broadcast

            nc.sync.dma_start(out=fs[:], in_=featp[:])
            nc.sync.dma_start(out=fds[:], in_=featd[:])
            nc.sync.dma_start(out=cts[:], in_=ctp[:])
            nc.sync.dma_start(out=ws[:], in_=wp[:])
            nc.sync.dma_start(out=qcs[:], in_=qcp[:])
            nc.sync.dma_start(out=bcs[:], in_=bcp[:])

            make_identity(nc, ident[:])
            nc.vector.memset(hid[:], 1.0)
            nc.vector.tensor_copy(q2b[:], qcs[:, 1:2].to_broadcast([128, 128]))

            # hidden^T, hidden, (a,bv) for all 17 node blocks
            for nb in range(NB):
                pt = ps.tile([128, 128], f32, tag="pp")
                nc.tensor.transpose(pt[:], fs[:, nb, :], ident[:])
                nc.vector.tensor_copy(ft[:], pt[:])
                ph = ps.tile([128, 128], f32, tag="pp")
                nc.tensor.matmul(ph[:], lhsT=ws[:], rhs=ft[:], start=True, stop=True)
                nc.vector.tensor_scalar_add(ht[:, nb, :], ph[:], bcs[:])
                ph2 = ps.tile([128, 128], f32, tag="pp")
                nc.tensor.transpose(ph2[:], ht[:, nb, :], ident[:])
                nc.scalar.activation(hid[:, nb, 0:128], ph2[:], AF.Copy)
                pab = ps.tile([128, 2], f32, tag="pp")
                nc.tensor.matmul(pab[:], lhsT=ht[:, nb, :], rhs=qcs[:], start=True, stop=True)
                nc.vector.tensor_copy(ab[:, nb, :], pab[:])
            nc.vector.tensor_scalar(ab02[:], ab[:], NEG, None, OP.mult)

            # per-core dst blocks: hidden_dst^T and bv row (partition-bcast)
            for j in range(DSTB):
                pt = ps.tile([128, 128], f32, tag="pp")
                nc.tensor.transpose(pt[:], fds[:, j, :], ident[:])
                nc.vector.tensor_copy(ft[:], pt[:])
                ph = ps.tile([128, 128], f32, tag="pp")
                nc.tensor.matmul(ph[:], lhsT=ws[:], rhs=ft[:], start=True, stop=True)
                nc.vector.tensor_scalar_add(hdt[:, j, :], ph[:], bcs[:])
                pbv = ps.tile([128, 128], f32, tag="pp")
                nc.tensor.matmul(pbv[:], lhsT=q2b[:], rhs=hdt[:, j, :], start=True, stop=True)
                nc.scalar.activation(bvb[:, 128 * j:128 * j + 128], pbv[:], AF.Copy)

            # main loop: W[s,d] = ct * exp(leaky(a[s]+bv[d])); out/S matmuls
            po = [psa.tile([128, 129], f32, tag=f"po{j}", name=f"po{j}")
                  for j in range(DSTB)]
            for S in range(NB):
                t1 = sc.tile([128, DSTB * 128], f32, tag="t1")
                t2 = sc.tile([128, DSTB * 128], f32, tag="t2")
                wg = sc.tile([128, DSTB * 128], f32, tag="wg")
                nc.scalar.activation(t1[:], bvb[:], AF.Exp, bias=ab[:, S, 0:1])
                nc.scalar.activation(t2[:], bvb[:], AF.Exp, bias=ab02[:, S, 0:1],
                                     scale=NEG)
                nc.vector.tensor_tensor(t1[:], t1[:], t2[:], op=OP.max)
                nc.vector.tensor_tensor(wg[:], t1[:], cts[:, S, :], op=OP.mult)
                for j in range(DSTB):
                    nc.tensor.matmul(po[j][:], lhsT=wg[:, 128 * j:128 * j + 128],
                                     rhs=hid[:, S, 0:129],
                                     start=(S == 0), stop=(S == NB - 1))

            for j in range(DSTB):
                sden = sc.tile([128, 1], f32, tag="sden")
                rec = sc.tile([128, 1], f32, tag="rec")
                osb = sc.tile([128, 128], f32, tag="osb")
                nc.vector.tensor_scalar_add(sden[:], po[j][:, 128:129], EPS)
                nc.vector.reciprocal(rec[:], sden[:])
                nc.scalar.activation(osb[:], po[j][:, 0:128], AF.Relu, scale=rec[:])
                nc.sync.dma_start(out=outp[:, j, :], in_=osb[:])

    nc.compile()
    return nc


def _host_counts(edge_list):
    """C_T[src, dst] edge multiplicities + self-loops, fp32."""
    e = np.asarray(edge_list)
    key = e[:, 0].astype(np.int64) * N + e[:, 1].astype(np.int64)
    cnt = np.bincount(key, minlength=N * N).astype(np.float32)
    cnt[np.arange(N, dtype=np.int64) * (N + 1)] += 1.0
    return cnt.reshape(N, N)


def kernel(node_feat_protein, node_feat_ligand, edge_list, W, b, query):
    _ensure_path()
    from concourse.bass_utils import run_bass_kernel_spmd

    feat = np.concatenate([
        np.asarray(node_feat_protein, dtype=np.float32),
        np.asarray(node_feat_ligand, dtype=np.float32),
    ])                                                     # [2176, 128]
    ct = _host_counts(edge_list)                           # [2176, 2176] src-major
    Wf = np.ascontiguousarray(np.asarray(W, dtype=np.float32))
    qc = np.ascontiguousarray(
        np.asarray(query, dtype=np.float32).reshape(2, 128).T)   # [128, 2]
    bc = np.ascontiguousarray(np.asarray(b, dtype=np.float32).reshape(128, 1))

    featp = np.ascontiguousarray(
        feat.reshape(NB, 128, 128).transpose(1, 0, 2))     # [128, 17, 128]
    ctb = ct.reshape(NB, 128, N)                           # [17, 128p, 2176]

    zero_blk = np.zeros((128, 128), np.float32)
    zero_ct = np.zeros((NB, 128, 128), np.float32)
    in_maps = []
    for c in range(NCORES):
        blocks = CORE_BLOCKS[c]
        fd = np.stack([feat[128 * gb:128 * gb + 128] if gb >= 0 else zero_blk
                       for gb in blocks])                  # [3, 128, 128]
        featd = np.ascontiguousarray(fd.transpose(1, 0, 2))
        ctc = np.stack([ctb[:, :, 128 * gb:128 * gb + 128] if gb >= 0 else zero_ct
                        for gb in blocks])                 # [3, 17, 128p, 128]
        ctc = np.ascontiguousarray(
            ctc.transpose(2, 1, 0, 3).reshape(128, NB, DSTB * 128))
        in_maps.append({"featp": featp, "featd": featd, "ctp": ctc,
                        "wp": Wf, "qcp": qc, "bcp": bc})

    nc = _program()
    res = run_bass_kernel_spmd(nc, in_maps, list(range(NCORES))).results

    out_full = np.zeros((N, D), np.float32)
    for c in range(NCORES):
        o = np.asarray(res[c]["outp"]).transpose(1, 0, 2)  # [3, 128, 128]
        for j, gb in enumerate(CORE_BLOCKS[c]):
            if gb >= 0:
                out_full[128 * gb:128 * gb + 128] = o[j]
    return out_full[:N_P], out_full[N_P:]


# revision 6
# speedup vs baseline: 1.0188x; 1.0188x over previous
"""GAT (single-head) message-passing kernel for Trainium2, 8 NeuronCores.

Math: with hidden = feat @ W + b, q1 = query[0,:128], q2 = query[0,128:],
per-edge logit l(s,d) = q1.hidden[s] + q2.hidden[d] = a[s] + bv[d] is
separable, so attention weights depend only on the (src,dst) pair.  The
whole edge aggregation collapses to dense linear algebra:
    out[d] = relu( (sum_s C[s,d] * P[s,d] * hidden[s]) / (S[d]+eps) )
    S[d]   = sum_s C[s,d] * P[s,d],   P[s,d] = exp(leaky_relu(a[s]+bv[d]))
where C is the (src,dst) edge-count matrix (self-loops folded in).
exp(leaky_relu(x)) is computed as max(exp(x), exp(0.2x)) (exp monotonic).
No per-segment max subtraction is needed: logits are O(10), exp cannot
overflow fp32, and the eps perturbation is ~1e-10 relative.

Sharding: dst blocks of 128 are round-robin assigned to the 8 cores
(core 0: blocks 0,8,16; core c: blocks c, c+8, pad).  Every core
computes hidden for all 2176 nodes (cheap) and the weighted matmul for
its own dst blocks only.  C is built on host (bincount over edge keys)
and shipped pre-sharded; node features/params are replicated.
"""

import functools
import sys

import numpy as np

N_P, N_L = 2048, 128
N = N_P + N_L            # 2176 nodes
D = 128
NB = N // 128            # 17 src blocks
DSTB = 3                 # dst blocks per core (padded)
NCORES = 8
EPS = 1e-10
NEG = 0.2

# core c -> global dst block ids (-1 = zero pad)
CORE_BLOCKS = [[0, 8, 16]] + [[c, c + 8, -1] for c in range(1, NCORES)]


def _ensure_path():
    try:
        import concourse.bass  # noqa: F401
    except ImportError:
        sys.path.insert(0, "/opt/trn_rl_repo")


@functools.lru_cache(maxsize=1)
def _program():
    _ensure_path()
    import concourse.bacc as bacc
    import concourse.tile as tile
    from concourse import mybir
    from concourse.masks import make_identity

    f32 = mybir.dt.float32
    AF = mybir.ActivationFunctionType
    OP = mybir.AluOpType

    nc = bacc.Bacc(None, target_bir_lowering=False, debug=False)

    featp = nc.dram_tensor("featp", [128, NB, 128], f32, kind="ExternalInput")
    featd = nc.dram_tensor("featd", [128, DSTB, 128], f32, kind="ExternalInput")
    ctp = nc.dram_tensor("ctp", [128, NB, DSTB * 128], f32, kind="ExternalInput")
    wp = nc.dram_tensor("wp", [128, 128], f32, kind="ExternalInput")
    qcp = nc.dram_tensor("qcp", [128, 2], f32, kind="ExternalInput")
    bcp = nc.dram_tensor("bcp", [128, 1], f32, kind="ExternalInput")
    outp = nc.dram_tensor("outp", [128, DSTB, 128], f32, kind="ExternalOutput")

    with tile.TileContext(nc) as tc:
        with tc.tile_pool(name="sb", bufs=1) as sb, \
             tc.tile_pool(name="sc", bufs=2) as sc, \
             tc.tile_pool(name="ps", bufs=3, space="PSUM") as ps, \
             tc.tile_pool(name="psa", bufs=1, space="PSUM") as psa:

            fs = sb.tile([128, NB, 128], f32, tag="fs")
            fds = sb.tile([128, DSTB, 128], f32, tag="fds")
            cts = sb.tile([128, NB, DSTB * 128], f32, tag="cts")
            ws = sb.tile([128, 128], f32, tag="ws")
            qcs = sb.tile([128, 2], f32, tag="qcs")
            bcs = sb.tile([128, 1], f32, tag="bcs")
            ident = sb.tile([128, 128], f32, tag="ident")
            ft = sb.tile([128, 128], f32, tag="ft")        # scratch feat^T block
            ht = sb.tile([128, NB, 128], f32, tag="ht")    # hidden^T blocks
            hdt = sb.tile([128, DSTB, 128], f32, tag="hdt")
            hid = sb.tile([128, NB, 129], f32, tag="hid")  # hidden blocks + ones col
            ab = sb.tile([128, NB, 2], f32, tag="ab")      # (a, bv) per node
            ab02 = sb.tile([128, NB, 2], f32, tag="ab02")  # 0.2 * (a, bv)
            bvb = sb.tile([128, DSTB * 128], f32, tag="bvb")
            q2b = sb.tile([128, 128], f32, tag="q2b")      # q2 col broadcast

            nc.sync.dma_start(out=fs[:], in_=featp[:])
            nc.sync.dma_start(out=fds[:], in_=featd[:])
            nc.sync.dma_start(out=cts[:], in_=ctp[:])
            nc.sync.dma_start(out=ws[:], in_=wp[:])
            nc.sync.dma_start(out=qcs[:], in_=qcp[:])
            nc.sync.dma_start(out=bcs[:], in_=bcp[:])

            make_identity(nc, ident[:])
            nc.vector.memset(hid[:], 1.0)
            nc.vector.tensor_copy(q2b[:], qcs[:, 1:2].to_broadcast([128, 128]))

            # hidden^T, hidden, (a,bv) for all 17 node blocks
            for nb in range(NB):
                pt = ps.tile([128, 128], f32, tag="pp")
                nc.tensor.transpose(pt[:], fs[:, nb, :], ident[:])
                nc.vector.tensor_copy(ft[:], pt[:])
                ph = ps.tile([128, 128], f32, tag="pp")
                nc.tensor.matmul(ph[:], lhsT=ws[:], rhs=ft[:], start=True, stop=True)
                nc.vector.tensor_scalar_add(ht[:, nb, :], ph[:], bcs[:])
                ph2 = ps.tile([128, 128], f32, tag="pp")
                nc.tensor.transpose(ph2[:], ht[:, nb, :], ident[:])
                nc.scalar.activation(hid[:, nb, 0:128], ph2[:], AF.Copy)
                pab = ps.tile([128, 2], f32, tag="pp")
                nc.tensor.matmul(pab[:], lhsT=ht[:, nb, :], rhs=qcs[:], start=True, stop=True)
                nc.vector.tensor_copy(ab[:, nb, :], pab[:])
            nc.vector.tensor_scalar(ab02[:], ab[:], NEG, None, OP.mult)

            # per-core dst blocks: hidden_dst^T and bv row (partition-bcast)
            for j in range(DSTB):
                pt = ps.tile([128, 128], f32, tag="pp")
                nc.tensor.transpose(pt[:], fds[:, j, :], ident[:])
                nc.vector.tensor_copy(ft[:], pt[:])
                ph = ps.tile([128, 128], f32, tag="pp")
                nc.tensor.matmul(ph[:], lhsT=ws[:], rhs=ft[:], start=True, stop=True)
                nc.vector.tensor_scalar_add(hdt[:, j, :], ph[:], bcs[:])
                pbv = ps.tile([128, 128], f32, tag="pp")
                nc.tensor.matmul(pbv[:], lhsT=q2b[:], rhs=hdt[:, j, :], start=True, stop=True)
                nc.scalar.activation(bvb[:, 128 * j:128 * j + 128], pbv[:], AF.Copy)

            # main loop: W[s,d] = ct * exp(leaky(a[s]+bv[d])); out/S matmuls
            po = [psa.tile([128, 129], f32, tag=f"po{j}", name=f"po{j}")
                  for j in range(DSTB)]
            for S in range(NB):
                t1 = sc.tile([128, DSTB * 128], f32, tag="t1")
                t2 = sc.tile([128, DSTB * 128], f32, tag="t2")
                wg = sc.tile([128, DSTB * 128], f32, tag="wg")
                nc.scalar.activation(t1[:], bvb[:], AF.Exp, bias=ab[:, S, 0:1])
                nc.scalar.activation(t2[:], bvb[:], AF.Exp, bias=ab02[:, S, 0:1],
                                     scale=NEG)
                nc.vector.tensor_tensor(t1[:], t1[:], t2[:], op=OP.max)
                nc.vector.tensor_tensor(wg[:], t1[:], cts[:, S, :], op=OP.mult)
                for j in range(DSTB):
                    nc.tensor.matmul(po[j][:], lhsT=wg[:, 128 * j:128 * j + 128],
                                     rhs=hid[:, S, 0:129],
                                     start=(S == 0), stop=(S == NB - 1))

            for j in range(DSTB):
                sden = sc.tile([128, 1], f32, tag="sden")
                rec = sc.tile([128, 1], f32, tag="rec")
                osb = sc.tile([128, 128], f32, tag="osb")
                nc.vector.tensor_scalar_add(sden[:], po[j][:, 128:129], EPS)
                nc.vector.reciprocal(rec[:], sden[:])
                nc.scalar.activation(osb[:], po[j][:, 0:128], AF.Relu, scale=rec[:])
                nc.sync.dma_start(out=outp[:, j, :], in_=osb[:])

    nc.compile()
    return nc


def _host_counts(edge_list):
    """C_T[src, dst] edge multiplicities + self-loops, fp32."""
    e = np.asarray(edge_list)
    key = e[:, 0].astype(np.int64) * N + e[:, 1].astype(np.int64)
    cnt = np.bincount(key, minlength=N * N).astype(np.float32)
    cnt[np.arange(N, dtype=np.int64) * (N + 1)] += 1.0
    return cnt.reshape(N, N)


def kernel(node_feat_protein, node_feat_ligand, edge_list, W, b, query):
    _ensure_path()
    from concourse.bass_utils import run_bass_kernel_spmd

    feat = np.concatenate([
        np.asarray(node_feat_protein, dtype=np.float32),
        np.asarray(node_feat_ligand, dtype=np.float32),
    ])                                                     # [2176, 128]
    ct = _host_counts(edge_list)                           # [2176, 2176] src-major
    Wf = np.ascontiguousarray(np.asarray(W, dtype=np.float32))
    qc = np.ascontiguousarray(
        np.asarray(query, dtype=np.float32).reshape(2, 128).T)   # [128, 2]
    bc = np.ascontiguousarray(np.asarray(b, dtype=np.float32).reshape(128, 1))

    featp = np.ascontiguousarray(
        feat.reshape(NB, 128, 128).transpose(1, 0, 2))     # [128, 17, 128]
    ctb = ct.reshape(NB, 128, N)                           # [17, 128p, 2176]

    zero_blk = np.zeros((128, 128), np.float32)
    zero_ct = np.zeros((NB, 128, 128), np.float32)
    in_maps = []
    for c in range(NCORES):
        blocks = CORE_BLOCKS[c]
        fd = np.stack([feat[128 * gb:128 * gb + 128] if gb >= 0 else zero_blk
                       for gb in blocks])                  # [3, 128, 128]
        featd = np.ascontiguousarray(fd.transpose(1, 0, 2))
        ctc = np.stack([ctb[:, :, 128 * gb:128 * gb + 128] if gb >= 0 else zero_ct
                        for gb in blocks])                 # [3, 17, 128p, 128]
        ctc = np.ascontiguousarray(
            ctc.transpose(2, 1, 0, 3).reshape(128, NB, DSTB * 128))
        in_maps.append({"featp": featp, "featd": featd, "ctp": ctc,
                        "wp": Wf, "qcp": qc, "bcp": bc})

    import os
    nc = _program()
    trace = bool(os.environ.get("BASS_PROFILE"))
    kw = {}
    if trace:
        kw = {"trace": True, "tmpdir": os.environ.get("BASS_TRACE_DIR") or None}
    kr = run_bass_kernel_spmd(nc, in_maps, list(range(NCORES)), **kw)
    if trace:
        globals()["LAST_EXEC_NS"] = kr.exec_time_ns
    res = kr.results

    out_full = np.zeros((N, D), np.float32)
    for c in range(NCORES):
        o = np.asarray(res[c]["outp"]).transpose(1, 0, 2)  # [3, 128, 128]
        for j, gb in enumerate(CORE_BLOCKS[c]):
            if gb >= 0:
                out_full[128 * gb:128 * gb + 128] = o[j]
    return out_full[:N_P], out_full[N_P:]
